# revision 16
# baseline (speedup 1.0000x reference)
"""Channel-attention module kernel for 8 Trainium2 NeuronCores.

reference semantics (B=2, C=128, N=D*H*W=147456):
    q = x.reshape(B, C, N)
    energy = q @ q^T                  # [B, C, C]
    attn = softmax(rowmax(energy) - energy, axis=-1)
          = softmax(-energy, axis=-1)             (rowmax shift is a no-op)
    out = attn @ q
    return x + gamma * out

Sharding: sequence-parallel over N. Core r owns columns
[r*N/8, (r+1)*N/8) of q for both batches. Each core computes a partial
energy (contraction over its local n), a per-batch AllReduce sums the
tiny [C, C] energy across the 8 cores, each core then computes the
softmax redundantly and applies the attention to its local columns.

Precision: fp16 end-to-end (host-measured rel err 1.5e-3 vs the 2e-2
gate; fp16 energy error std ~0.11 vs a minimum softmax argmin gap of
0.03 on these inputs -- no argmin flips). This makes the energy
matmuls 4x cheaper than fp32 (1 PE cycle/row vs 4) and halves DMA.

Key layout trick: the host passes TWO fp16 views of q:
  - qt [B, 128, NLOC]: packed pre-transposed tiles,
    qt[b, p, t*128 + c] = q[b, c, t*128 + p]. Tile t sits at free
    columns [t*128, (t+1)*128) as an [n=128, c=128] matmul operand, so
    phase 1 needs ZERO on-chip transposes (the old kernel spent ~60% of
    its tensor-engine time on per-tile fp32 transposes + their
    LDWEIGHTS + DVE copy-backs).
  - q [B, C, NLOC]: natural layout, streamed 512-wide in phase 2.
Both loads are fully contiguous (4KB per partition line).

Residual is folded into the attention matrix (attn_s = gamma/Z * P + I;
P's diagonal is exactly 0 because the energy diagonal ~ +N dominates),
so phase 2 is out = attn_s @ q and no fp32 x is ever needed on chip.

Stores go out on the scalar HWDGE ring so they don't FIFO-queue behind
the phase-2 loads on the sync ring.
"""

import sys

sys.path.insert(0, "/opt/trn_rl_repo")

import numpy as np

B, C = 2, 128
D, H, W = 16, 96, 96
N = D * H * W  # 147456
NCORES = 8
NLOC = N // NCORES  # 18432
CHUNK = 2048
NCHUNK = NLOC // CHUNK  # 9
NTILE_C = CHUNK // C  # 16 tiles per chunk
NT = NLOC // C  # 144 tiles per batch
OTILE = 512

_compiled = {}


def _log(msg):
    import time as _t
    print(f"[kernel {_t.strftime('%H:%M:%S')}] {msg}", flush=True)


def _build():
    import concourse.bacc as bacc
    import concourse.tile as tile
    import concourse.mybir as mybir

    _log("build start")

    f32 = mybir.dt.float32
    f16 = mybir.dt.float16
    nc = bacc.Bacc("TRN2", target_bir_lowering=False, debug=False,
                   num_devices=NCORES)

    qt_d = nc.dram_tensor("qt", [B, C, NLOC], f16, kind="ExternalInput").ap()
    q_d = nc.dram_tensor("q", [B, C, NLOC], f16, kind="ExternalInput").ap()
    g_d = nc.dram_tensor("gamma_col", [C, 1], f32, kind="ExternalInput").ap()
    id_d = nc.dram_tensor("ident", [C, C], f32, kind="ExternalInput").ap()
    o_d = nc.dram_tensor("out", [B, C, NLOC], f16, kind="ExternalOutput").ap()

    with tile.TileContext(nc) as tc:
        with (
            tc.tile_pool(name="qt", bufs=B * NCHUNK) as qtp,
            tc.tile_pool(name="q", bufs=B * NCHUNK) as qp,
            tc.tile_pool(name="eps", bufs=1, space="PSUM") as eps,
            tc.tile_pool(name="tps", bufs=1, space="PSUM") as tps,
            tc.tile_pool(name="ops", bufs=6, space="PSUM") as ops,
            tc.tile_pool(name="misc", bufs=1) as mp,
            tc.tile_pool(name="ost", bufs=3) as ostp,
            tc.tile_pool(name="dram", bufs=1, space="DRAM") as dramp,
        ):
            ident = mp.tile([C, C], f32, name="ident_sb")
            nc.sync.dma_start(ident[:], id_d[:])
            gcol = mp.tile([C, 1], f32, name="gcol")
            nc.sync.dma_start(gcol[:], g_d[:])

            # Warm-up collective: the FIRST collective pays a huge cold
            # start. hw-measured: with this dummy fired at t~1us it
            # completes at t=78us and the real ARs land at 95/107us;
            # WITHOUT it, AR(b0) does not complete until t=232us. Keep it.
            w_in = dramp.tile([1, 1], f32, name="w_in")
            w_out = dramp.tile([1, 1], f32, name="w_out", addr_space="Shared")
            nc.gpsimd.dma_start(w_in[:], gcol[0:1, :])
            nc.gpsimd.collective_compute(
                "AllReduce", mybir.AluOpType.add,
                replica_groups=[list(range(NCORES))],
                ins=[w_in.opt()], outs=[w_out.opt()],
            )

            # All input loads on the sync ring, in consumption order:
            # qt(b0), qt(b1) feed phase 1; q(b0), q(b1) feed phase 2
            # (needed only after the per-batch AllReduce returns).
            qt_sb = [[None] * NCHUNK for _ in range(B)]
            q_sb = [[None] * NCHUNK for _ in range(B)]
            for b in range(B):
                for k in range(NCHUNK):
                    t = qtp.tile([C, CHUNK], f16, name=f"qt_{b}_{k}", tag="qt")
                    nc.sync.dma_start(t[:], qt_d[b, :, k * CHUNK:(k + 1) * CHUNK])
                    qt_sb[b][k] = t
            for b in range(B):
                for k in range(NCHUNK):
                    t = qp.tile([C, CHUNK], f16, name=f"q_{b}_{k}", tag="q")
                    nc.sync.dma_start(t[:], q_d[b, :, k * CHUNK:(k + 1) * CHUNK])
                    q_sb[b][k] = t

            # ---- phase 1: local energy; ONE combined AllReduce ----
            # Both batches' partial energies ride in a single [C, 2C] AR:
            # each CC op costs ~10.5us warm, so merging saves ~10us, and a
            # single completion event means the two softmaxes can never
            # head-of-line-block each other on DVE.
            e_cat = mp.tile([C, 2 * C], f32, name="e_cat")
            for b in range(B):
                e_ps = eps.tile([C, C], f32, name=f"e_ps{b}", tag="e")
                t = 0
                for k in range(NCHUNK):
                    xt = qt_sb[b][k]
                    for j in range(NTILE_C):
                        sl = xt[:, j * C:(j + 1) * C]
                        nc.tensor.matmul(e_ps[:], sl, sl,
                                         start=(t == 0), stop=(t == NT - 1))
                        t += 1
                nc.vector.tensor_copy(e_cat[:, b * C:(b + 1) * C], e_ps[:])

            ar_in = dramp.tile([C, 2 * C], f32, name="ar_in")
            ar_out = dramp.tile([C, 2 * C], f32, name="ar_out",
                                addr_space="Shared")
            # bounce DMAs on GPSIMD/SWDGE: the HWDGE (sync) ring is
            # strictly FIFO, so a collective-gated load there would
            # block all later chunk loads.
            nc.gpsimd.dma_start(ar_in[:], e_cat[:])
            nc.gpsimd.collective_compute(
                "AllReduce", mybir.AluOpType.add,
                replica_groups=[list(range(NCORES))],
                ins=[ar_in.opt()], outs=[ar_out.opt()],
            )
            # read the reduced energy back on the sync HWDGE ring: the SWDGE
            # drain after a gpsimd read adds ~2us before the completion
            # semaphore fires. The sync ring is empty at this point (all
            # input loads done ~70us; stores are emitted later in the FIFO).
            e_red = mp.tile([C, 2 * C], f32, name="e_red")
            nc.sync.dma_start(e_red[:], ar_out[:])
            E_sb = [e_red[:, 0:C], e_red[:, C:2 * C]]

            # PE p-state pre-warm: the tensor engine idles ~45us waiting for
            # the AllReduce and drops out of max clock (needs ~3us of
            # continuous work to ramp back to 2.4GHz; at 1.2GHz the phase-2
            # matmuls run 634ns instead of ~315ns). Run ~3.5us of dummy
            # matmuls on e_red as soon as it lands -- the DVE/ACT softmax
            # chain runs concurrently, so this PE time is free.
            warm_ps = eps.tile([C, C], f32, name="warm_ps", tag="e")
            for w in range(8):
                nc.tensor.matmul(warm_ps[:], e_red[:, 0:C], e_red[:, 0:C],
                                 start=True, stop=True)

            # ---- phase 2: softmax + apply, per batch ----
            def emit_softmax(b):
                E_b = E_sb[b]
                mcol = mp.tile([C, 1], f32, name=f"mcol{b}")
                nc.vector.tensor_reduce(mcol[:], E_b, axis=mybir.AxisListType.X,
                                        op=mybir.AluOpType.min)
                P_b = mp.tile([C, C], f32, name=f"P{b}")
                zcol = mp.tile([C, 1], f32, name=f"zcol{b}")
                # P = exp(min_row - E), zcol = rowsum(P); exponents <= 0.
                # P's diagonal is exp(min - ~+18000*8) == 0 exactly.
                nc.scalar.activation(P_b[:], E_b,
                                     mybir.ActivationFunctionType.Exp,
                                     bias=mcol[:], scale=-1.0,
                                     accum_out=zcol[:])
                rz = mp.tile([C, 1], f32, name=f"rz{b}")
                nc.vector.reciprocal(rz[:], zcol[:])
                scol = mp.tile([C, 1], f32, name=f"scol{b}")
                nc.vector.tensor_tensor(scol[:], rz[:], gcol[:],
                                        op=mybir.AluOpType.mult)
                # attn_s = (gamma/Z) * P + I  -> matmul computes x + gamma*attn@q
                nc.vector.tensor_scalar_mul(P_b[:], P_b[:], scol[:])
                nc.vector.tensor_add(P_b[:], P_b[:], ident[:])
                tp2 = tps.tile([C, C], f32, name=f"tpP{b}", tag="tp")
                nc.tensor.transpose(tp2[:], P_b[:], ident[:])
                attnT = mp.tile([C, C], f16, name=f"attnT{b}")
                nc.vector.tensor_copy(attnT[:], tp2[:])  # fp32 psum -> fp16
                return attnT

            def emit_apply_chunk(b, attnT, k):
                ost = ostp.tile([C, CHUNK], f16, name=f"ost_{b}_{k}",
                                tag="ost")
                for j in range(CHUNK // OTILE):
                    op = ops.tile([C, OTILE], f32, name=f"op_{b}_{k}_{j}",
                                  tag="op")
                    nc.tensor.matmul(
                        op[:], attnT[:],
                        q_sb[b][k][:, j * OTILE:(j + 1) * OTILE],
                        start=True, stop=True)
                    dst = ost[:, j * OTILE:(j + 1) * OTILE]
                    # split PSUM->SBUF copies across DVE and ACT; gpsimd has
                    # no PSUM port. Softmax(b1)'s few DVE ops queue behind at
                    # most a couple of copies (~1.5us), acceptable.
                    if j % 2 == 0:
                        nc.vector.tensor_copy(dst, op[:])
                    else:
                        nc.scalar.copy(dst, op[:])
                # sync ring: idle during phase 2 (all loads done by ~70us,
                # phase 2 starts ~108us), and keeping stores off the scalar
                # SEQ leaves it free for its share of the PSUM copies.
                nc.sync.dma_start(o_d[b, :, k * CHUNK:(k + 1) * CHUNK],
                                  ost[:])

            attnTs = [emit_softmax(b) for b in range(B)]
            for b in range(B):
                for k in range(NCHUNK):
                    emit_apply_chunk(b, attnTs[b], k)

    _log("tile context done; bacc compile start")
    nc.compile()
    _log("bacc compile done")
    return nc


def _get_nc():
    if "nc" not in _compiled:
        _compiled["nc"] = _build()
    return _compiled["nc"]


def kernel(x, gamma, _trace=False, _tmpdir=None):
    from concourse import bass_utils

    x = np.asarray(x)
    gamma = np.asarray(gamma, dtype=np.float32)
    qf = x.reshape(B, C, N).astype(np.float16)
    gcol = np.full((C, 1), gamma[0], dtype=np.float32)
    ident = np.eye(C, dtype=np.float32)

    in_maps = []
    for r in range(NCORES):
        qloc = qf[:, :, r * NLOC:(r + 1) * NLOC]
        # qt[b, p, t*128 + c] = qloc[b, c, t*128 + p]
        qtl = np.ascontiguousarray(
            qloc.reshape(B, C, NT, C).transpose(0, 3, 2, 1).reshape(B, C, NLOC))
        in_maps.append({
            "qt": qtl,
            "q": np.ascontiguousarray(qloc),
            "gamma_col": gcol,
            "ident": ident,
        })

    nc = _get_nc()
    _log("launching run_bass_kernel_spmd")
    res = bass_utils.run_bass_kernel_spmd(
        nc, in_maps, core_ids=list(range(NCORES)), trace=_trace,
        tmpdir=_tmpdir)
    outs = [res.results[r]["out"] for r in range(NCORES)]
    full = np.concatenate(outs, axis=2).astype(np.float32)
    full = full.reshape(B, C, D, H, W)
    if _trace:
        return full, res
    return full


# revision 18
# speedup vs baseline: 1.0050x; 1.0050x over previous
"""Channel-attention module kernel for 8 Trainium2 NeuronCores.

reference semantics (B=2, C=128, N=D*H*W=147456):
    q = x.reshape(B, C, N)
    energy = q @ q^T                  # [B, C, C]
    attn = softmax(rowmax(energy) - energy, axis=-1)
          = softmax(-energy, axis=-1)             (rowmax shift is a no-op)
    out = attn @ q
    return x + gamma * out

Sharding: sequence-parallel over N. Core r owns columns
[r*N/8, (r+1)*N/8) of q for both batches. Each core computes a partial
energy (contraction over its local n), a per-batch AllReduce sums the
tiny [C, C] energy across the 8 cores, each core then computes the
softmax redundantly and applies the attention to its local columns.

Precision: fp16 end-to-end (host-measured rel err 1.5e-3 vs the 2e-2
gate; fp16 energy error std ~0.11 vs a minimum softmax argmin gap of
0.03 on these inputs -- no argmin flips). This makes the energy
matmuls 4x cheaper than fp32 (1 PE cycle/row vs 4) and halves DMA.

Key layout trick: the host passes TWO fp16 views of q:
  - qt [B, 128, NLOC]: packed pre-transposed tiles,
    qt[b, p, t*128 + c] = q[b, c, t*128 + p]. Tile t sits at free
    columns [t*128, (t+1)*128) as an [n=128, c=128] matmul operand, so
    phase 1 needs ZERO on-chip transposes (the old kernel spent ~60% of
    its tensor-engine time on per-tile fp32 transposes + their
    LDWEIGHTS + DVE copy-backs).
  - q [B, C, NLOC]: natural layout, streamed 512-wide in phase 2.
Both loads are fully contiguous (4KB per partition line).

Residual is folded into the attention matrix (attn_s = gamma/Z * P + I;
P's diagonal is exactly 0 because the energy diagonal ~ +N dominates),
so phase 2 is out = attn_s @ q and no fp32 x is ever needed on chip.

Stores go out on the scalar HWDGE ring so they don't FIFO-queue behind
the phase-2 loads on the sync ring.
"""

import sys

sys.path.insert(0, "/opt/trn_rl_repo")

import numpy as np

B, C = 2, 128
D, H, W = 16, 96, 96
N = D * H * W  # 147456
NCORES = 8
NLOC = N // NCORES  # 18432
CHUNK = 2048
NCHUNK = NLOC // CHUNK  # 9
NTILE_C = CHUNK // C  # 16 tiles per chunk
NT = NLOC // C  # 144 tiles per batch
OTILE = 512

_compiled = {}


def _log(msg):
    import time as _t
    print(f"[kernel {_t.strftime('%H:%M:%S')}] {msg}", flush=True)


def _build():
    import concourse.bacc as bacc
    import concourse.tile as tile
    import concourse.mybir as mybir

    _log("build start")

    f32 = mybir.dt.float32
    f16 = mybir.dt.float16
    nc = bacc.Bacc("TRN2", target_bir_lowering=False, debug=False,
                   num_devices=NCORES)

    qt_d = nc.dram_tensor("qt", [B, C, NLOC], f16, kind="ExternalInput").ap()
    q_d = nc.dram_tensor("q", [B, C, NLOC], f16, kind="ExternalInput").ap()
    g_d = nc.dram_tensor("gamma_col", [C, 1], f32, kind="ExternalInput").ap()
    id_d = nc.dram_tensor("ident", [C, C], f32, kind="ExternalInput").ap()
    o_d = nc.dram_tensor("out", [B, C, NLOC], f16, kind="ExternalOutput").ap()

    with tile.TileContext(nc) as tc:
        with (
            tc.tile_pool(name="qt", bufs=B * NCHUNK) as qtp,
            tc.tile_pool(name="q", bufs=B * NCHUNK) as qp,
            tc.tile_pool(name="eps", bufs=1, space="PSUM") as eps,
            tc.tile_pool(name="tps", bufs=1, space="PSUM") as tps,
            tc.tile_pool(name="ops", bufs=6, space="PSUM") as ops,
            tc.tile_pool(name="misc", bufs=1) as mp,
            tc.tile_pool(name="ost", bufs=3) as ostp,
            tc.tile_pool(name="dram", bufs=1, space="DRAM") as dramp,
        ):
            ident = mp.tile([C, C], f32, name="ident_sb")
            nc.sync.dma_start(ident[:], id_d[:])
            gcol = mp.tile([C, 1], f32, name="gcol")
            nc.sync.dma_start(gcol[:], g_d[:])

            # Warm-up collective: the FIRST collective pays a huge cold
            # start. hw-measured: with this dummy fired at t~1us it
            # completes at t=78us and the real ARs land at 95/107us;
            # WITHOUT it, AR(b0) does not complete until t=232us. Keep it.
            # Bounce the warm-up input over the sync HWDGE ring (runs right
            # behind the ident/gcol loads, ~11us) instead of gpsimd SWDGE
            # whose post-DMA drain delayed the ncfw doorbell to ~24us.
            w_in = dramp.tile([1, 1], f32, name="w_in")
            w_out = dramp.tile([1, 1], f32, name="w_out", addr_space="Shared")
            nc.sync.dma_start(w_in[:], gcol[0:1, :])
            nc.gpsimd.collective_compute(
                "AllReduce", mybir.AluOpType.add,
                replica_groups=[list(range(NCORES))],
                ins=[w_in.opt()], outs=[w_out.opt()],
            )

            # All input loads on the sync ring, in consumption order:
            # qt(b0), qt(b1) feed phase 1; q(b0), q(b1) feed phase 2
            # (needed only after the per-batch AllReduce returns).
            qt_sb = [[None] * NCHUNK for _ in range(B)]
            q_sb = [[None] * NCHUNK for _ in range(B)]
            for b in range(B):
                for k in range(NCHUNK):
                    t = qtp.tile([C, CHUNK], f16, name=f"qt_{b}_{k}", tag="qt")
                    nc.sync.dma_start(t[:], qt_d[b, :, k * CHUNK:(k + 1) * CHUNK])
                    qt_sb[b][k] = t
            for b in range(B):
                for k in range(NCHUNK):
                    t = qp.tile([C, CHUNK], f16, name=f"q_{b}_{k}", tag="q")
                    nc.sync.dma_start(t[:], q_d[b, :, k * CHUNK:(k + 1) * CHUNK])
                    q_sb[b][k] = t

            # ---- phase 1: local energy; ONE combined AllReduce ----
            # Both batches' partial energies ride in a single [C, 2C] AR:
            # each CC op costs ~10.5us warm, so merging saves ~10us, and a
            # single completion event means the two softmaxes can never
            # head-of-line-block each other on DVE.
            e_cat = mp.tile([C, 2 * C], f32, name="e_cat")
            for b in range(B):
                e_ps = eps.tile([C, C], f32, name=f"e_ps{b}", tag="e")
                t = 0
                for k in range(NCHUNK):
                    xt = qt_sb[b][k]
                    for j in range(NTILE_C):
                        sl = xt[:, j * C:(j + 1) * C]
                        nc.tensor.matmul(e_ps[:], sl, sl,
                                         start=(t == 0), stop=(t == NT - 1))
                        t += 1
                nc.vector.tensor_copy(e_cat[:, b * C:(b + 1) * C], e_ps[:])

            ar_in = dramp.tile([C, 2 * C], f32, name="ar_in")
            ar_out = dramp.tile([C, 2 * C], f32, name="ar_out",
                                addr_space="Shared")
            # bounce DMAs on GPSIMD/SWDGE: the HWDGE (sync) ring is
            # strictly FIFO, so a collective-gated load there would
            # block all later chunk loads.
            nc.gpsimd.dma_start(ar_in[:], e_cat[:])
            nc.gpsimd.collective_compute(
                "AllReduce", mybir.AluOpType.add,
                replica_groups=[list(range(NCORES))],
                ins=[ar_in.opt()], outs=[ar_out.opt()],
            )
            # read the reduced energy back on the sync HWDGE ring: the SWDGE
            # drain after a gpsimd read adds ~2us before the completion
            # semaphore fires. The sync ring is empty at this point (all
            # input loads done ~70us; stores are emitted later in the FIFO).
            e_red = mp.tile([C, 2 * C], f32, name="e_red")
            nc.sync.dma_start(e_red[:], ar_out[:])
            E_sb = [e_red[:, 0:C], e_red[:, C:2 * C]]

            # PE p-state pre-warm: the tensor engine idles ~45us waiting for
            # the AllReduce and drops out of max clock (needs ~3us of
            # continuous work to ramp back to 2.4GHz; at 1.2GHz the phase-2
            # matmuls run 634ns instead of ~315ns). Run ~3.5us of dummy
            # matmuls on e_red as soon as it lands -- the DVE/ACT softmax
            # chain runs concurrently, so this PE time is free.
            warm_ps = eps.tile([C, C], f32, name="warm_ps", tag="e")
            for w in range(8):
                nc.tensor.matmul(warm_ps[:], e_red[:, 0:C], e_red[:, 0:C],
                                 start=True, stop=True)

            # ---- phase 2: softmax + apply, per batch ----
            def emit_softmax(b):
                E_b = E_sb[b]
                mcol = mp.tile([C, 1], f32, name=f"mcol{b}")
                nc.vector.tensor_reduce(mcol[:], E_b, axis=mybir.AxisListType.X,
                                        op=mybir.AluOpType.min)
                P_b = mp.tile([C, C], f32, name=f"P{b}")
                zcol = mp.tile([C, 1], f32, name=f"zcol{b}")
                # P = exp(min_row - E), zcol = rowsum(P); exponents <= 0.
                # P's diagonal is exp(min - ~+18000*8) == 0 exactly.
                nc.scalar.activation(P_b[:], E_b,
                                     mybir.ActivationFunctionType.Exp,
                                     bias=mcol[:], scale=-1.0,
                                     accum_out=zcol[:])
                rz = mp.tile([C, 1], f32, name=f"rz{b}")
                nc.vector.reciprocal(rz[:], zcol[:])
                scol = mp.tile([C, 1], f32, name=f"scol{b}")
                nc.vector.tensor_tensor(scol[:], rz[:], gcol[:],
                                        op=mybir.AluOpType.mult)
                # attn_s = (gamma/Z) * P + I  -> matmul computes x + gamma*attn@q
                nc.vector.tensor_scalar_mul(P_b[:], P_b[:], scol[:])
                nc.vector.tensor_add(P_b[:], P_b[:], ident[:])
                tp2 = tps.tile([C, C], f32, name=f"tpP{b}", tag="tp")
                nc.tensor.transpose(tp2[:], P_b[:], ident[:])
                attnT = mp.tile([C, C], f16, name=f"attnT{b}")
                nc.vector.tensor_copy(attnT[:], tp2[:])  # fp32 psum -> fp16
                return attnT

            def emit_apply_chunk(b, attnT, k):
                ost = ostp.tile([C, CHUNK], f16, name=f"ost_{b}_{k}",
                                tag="ost")
                for j in range(CHUNK // OTILE):
                    op = ops.tile([C, OTILE], f32, name=f"op_{b}_{k}_{j}",
                                  tag="op")
                    nc.tensor.matmul(
                        op[:], attnT[:],
                        q_sb[b][k][:, j * OTILE:(j + 1) * OTILE],
                        start=True, stop=True)
                    dst = ost[:, j * OTILE:(j + 1) * OTILE]
                    # split PSUM->SBUF copies across DVE and ACT; gpsimd has
                    # no PSUM port. Softmax(b1)'s few DVE ops queue behind at
                    # most a couple of copies (~1.5us), acceptable.
                    if j % 2 == 0:
                        nc.vector.tensor_copy(dst, op[:])
                    else:
                        nc.scalar.copy(dst, op[:])
                # sync ring: idle during phase 2 (all loads done by ~70us,
                # phase 2 starts ~108us), and keeping stores off the scalar
                # SEQ leaves it free for its share of the PSUM copies.
                nc.sync.dma_start(o_d[b, :, k * CHUNK:(k + 1) * CHUNK],
                                  ost[:])

            attnTs = [emit_softmax(b) for b in range(B)]
            for b in range(B):
                for k in range(NCHUNK):
                    emit_apply_chunk(b, attnTs[b], k)

    _log("tile context done; bacc compile start")
    nc.compile()
    _log("bacc compile done")
    return nc


def _get_nc():
    if "nc" not in _compiled:
        _compiled["nc"] = _build()
    return _compiled["nc"]


def kernel(x, gamma, _trace=False, _tmpdir=None):
    from concourse import bass_utils

    x = np.asarray(x)
    gamma = np.asarray(gamma, dtype=np.float32)
    qf = x.reshape(B, C, N).astype(np.float16)
    gcol = np.full((C, 1), gamma[0], dtype=np.float32)
    ident = np.eye(C, dtype=np.float32)

    in_maps = []
    for r in range(NCORES):
        qloc = qf[:, :, r * NLOC:(r + 1) * NLOC]
        # qt[b, p, t*128 + c] = qloc[b, c, t*128 + p]
        qtl = np.ascontiguousarray(
            qloc.reshape(B, C, NT, C).transpose(0, 3, 2, 1).reshape(B, C, NLOC))
        in_maps.append({
            "qt": qtl,
            "q": np.ascontiguousarray(qloc),
            "gamma_col": gcol,
            "ident": ident,
        })

    nc = _get_nc()
    _log("launching run_bass_kernel_spmd")
    res = bass_utils.run_bass_kernel_spmd(
        nc, in_maps, core_ids=list(range(NCORES)), trace=_trace,
        tmpdir=_tmpdir)
    outs = [res.results[r]["out"] for r in range(NCORES)]
    full = np.concatenate(outs, axis=2).astype(np.float32)
    full = full.reshape(B, C, D, H, W)
    if _trace:
        return full, res
    return full
